# revision 27
# baseline (speedup 1.0000x reference)
"""Dynamic (MoE-routed) 3x3 conv kernel for Trainium2, 8 NeuronCores.

Problem: nn_DynamicConv_670014898566
  x         [32, 64, 128, 128] f32
  w_route   [4, 64] f32
  b_route   [4] f32
  w_experts [4, 64, 64, 3, 3] f32
  y = per-sample conv2d(x, sigmoid(mean(x,HW) @ w_route.T + b_route) @ w_experts, SAME)

Sharding: data-parallel over batch, 4 samples per core (2 pairs of 2).

v4 design (vs the 127us baseline):
  - x is cast to bf16 on the host (numerically identical to the SDMA
    cast-DMA the baseline used; the conv consumed bf16 either way), so
    the device reads half the bytes
  - loads are split across BOTH HWDGE rings in parallel (each ring has
    a ~2us inter-DMA bubble; two rings overlap them and together reach
    the HBM ceiling)
  - constants are merged into 3 host-prepared DMAs (expert kernels
    pre-transposed/replicated to the full 128-partition lhsT layout,
    routing matrix pre-scaled and replicated, masks + bias packed)
  - per-chunk channel-sum reduces for pair 0 (DVE first half, ACT rest)
    fire as each chunk lands; ALL pair-1 reduces run on the otherwise
    idle GpSimd engine so they can never block the DVE/ACT pipelines
  - HAM warmup: pipelined dummy matmuls (alternating PSUM half-banks to
    avoid write-write serialization) gated on pair-0 chunks keep the PE
    at 2.4 GHz through conv start without clogging the PE FIFO
  - routing chain: tail sum -> masked cols -> logits matmul -> bias ->
    sigmoid -> broadcast matmul -> copy to SBUF -> 4-op DVE mix chain
  - conv: per (sample h, chunk-parity q) stream, 9 shifted bf16 matmuls
    accumulate into one PSUM region; 4-way PE tile parallelism
  - pair-1 routing tail spliced mid-conv-p0 with wide margins
  - y stored as bf16 in 262KB two-group DMAs (A-samples on sync ring,
    B-samples on scalar); host upcasts to f32
"""

import sys

sys.path.insert(0, "/opt/trn_rl_repo")

import numpy as np

B, C, H, W = 32, 64, 128, 128
E = 4
HW = H * W
N_CORES = 8
NS = B // N_CORES          # samples per core = 4
NPAIR = NS // 2            # pairs per core = 2
NT = 16                    # chunk-pairs (t) per pair
NSG = 2                    # store super-groups per pair (4 g of 2 t each)
# load chunks (cols of the [128, 16384] pair tile), all on the gpsimd
# SWDGE queue (the only path that streams near HBM rate; descriptor
# generation costs ~4.15us/DMA serially on Q7, so few chunks)
P0_CHUNKS = [(0, 6144), (6144, 6144), (12288, 3584), (15872, 512)]
P1_CHUNKS = [(0, 4096), (4096, 4096), (8192, 4096), (12288, 4096)]
# pair-0 chunk 0 rides the sync HWDGE ring (starts immediately, in
# parallel with the SWDGE stream); warmup groups keyed to arrival order
WU_ORDER = [(1, 18), (2, 10), (3, 8), (0, 10)]
# wait-ladder (ms, scheduler-sim time) spreading pair-1 reduce pieces and
# routing tail through the conv-p0 window so they never displace the
# PSUM-evacuation copies in the static engine queues
PIECE_W0 = 0.022
PIECE_DW = 0.0018
# full-coverage tap first (owns start=True so PSUM has_written covers the bank)
TAPS = [(1, 1), (0, 0), (0, 1), (0, 2), (1, 0), (1, 2), (2, 0), (2, 1), (2, 2)]

_CACHE = {}


def _build_nc():
    import concourse.bacc as bacc
    import concourse.mybir as mybir
    import concourse.tile as tile

    dt = mybir.dt
    f32 = dt.float32
    bf16 = dt.bfloat16

    nc = bacc.Bacc("TRN2", target_bir_lowering=False, debug=False, num_devices=N_CORES)

    x_d = nc.dram_tensor("x", [NS, C, H, W], bf16, kind="ExternalInput")
    # [128, 2304] bf16: lhsT expert kernels, host-replicated to both halves
    weT_d = nc.dram_tensor("weT128", [128, E * C * 9], bf16, kind="ExternalInput")
    # [128, 6] f32: cols 0:2 mask01, cols 2:6 wrT (pre-scaled, host-replicated)
    cA_d = nc.dram_tensor("constsA", [128, 6], f32, kind="ExternalInput")
    # [2, 132] f32: cols 0:128 mask2, cols 128:132 bias (both rows)
    cB_d = nc.dram_tensor("constsB", [2, 132], f32, kind="ExternalInput")
    y_d = nc.dram_tensor("y", [NS, C, H, W], bf16, kind="ExternalOutput")

    # x viewed as [(b c), (h w)]: pair p = rows 128p..128p+128
    x_flat = x_d.ap().rearrange("b c h w -> (b c) (h w)")
    # y viewed as [b, c, SG, g4, t2, parity, 4*W] for batched stores
    y_g = y_d.ap().rearrange(
        "b c (G g4 t2 hf r) w -> b c G g4 t2 hf (r w)", G=NSG, g4=4, t2=2, hf=2, r=4
    )

    with tile.TileContext(nc) as tc:
        with (
            tc.tile_pool(name="const", bufs=1) as cpool,
            tc.tile_pool(name="xp", bufs=2) as xpool,
            tc.tile_pool(name="mix", bufs=2) as mpool,
            tc.tile_pool(name="small", bufs=2) as spool,
            tc.tile_pool(name="stage", bufs=4) as stpool,
            tc.tile_pool(name="cps", bufs=7, space="PSUM") as convps,
            tc.tile_pool(name="rps", bufs=1, space="PSUM") as rps,
        ):
            xb_t = [
                xpool.tile([128, HW], bf16, tag="xt", name=f"xb_p{p}")
                for p in range(NPAIR)
            ]
            # pooled partials: p0 cols 0-8 (DVE/ACT per half-chunk), p1 cols
            # 0-16 (17 small pieces alternating DVE/ACT); col 17 = total
            pooled_t = [
                spool.tile([128, 18], f32, tag="pooled", name=f"pooled_{p}")
                for p in range(NPAIR)
            ]

            weT = cpool.tile([128, E * C * 9], bf16)
            constsA = cpool.tile([128, 6], f32)
            constsB = cpool.tile([2, 132], f32)
            mask01 = constsA[:, 0:2]
            wrT_rep = constsA[:, 2:6]
            mask2 = constsB[:, 0:128]
            bias_rep = constsB[:, 128:132]

            # ---------------- loads: SWDGE stream + one HWDGE chunk ----------
            def emit_load(p, c0, cn, eng=None):
                (eng or nc.gpsimd).dma_start(
                    xb_t[p][:, c0 : c0 + cn],
                    x_flat[128 * p : 128 * p + 128, c0 : c0 + cn],
                )

            # chunk 0 of pair 0 on the sync ring, concurrent with the SWDGE
            # stream carrying everything else
            emit_load(0, *P0_CHUNKS[0], eng=nc.sync)
            for c0, cn in P0_CHUNKS[1:]:
                emit_load(0, c0, cn)
            for c0, cn in P1_CHUNKS:
                emit_load(1, c0, cn)
            # constants on the scalar ring (idle until B-stores)
            nc.scalar.dma_start(weT[:], weT_d.ap())
            nc.scalar.dma_start(constsA[:], cA_d.ap())
            nc.scalar.dma_start(constsB[:], cB_d.ap())
            # sigmoid table warm so the routing sigmoid isn't delayed later
            sig_warm = cpool.tile([1, 1], f32)
            nc.scalar.activation(
                sig_warm[:], constsB[0:1, 128:129],
                mybir.ActivationFunctionType.Sigmoid,
            )

            # ---------------- reduces ---------------------------------------
            act_scratch = cpool.tile([128, 3072], bf16)

            def emit_red0(d):
                """Pair-0 chunk d channel-sums: DVE first half, ACT the rest."""
                c0, cn = P0_CHUNKS[d]
                half = min((cn // 2 + 511) // 512 * 512, cn)
                nc.vector.reduce_sum(
                    pooled_t[0][:, 2 * d : 2 * d + 1],
                    xb_t[0][:, c0 : c0 + half],
                    axis=mybir.AxisListType.X,
                )
                if cn > half:
                    nc.scalar.activation(
                        act_scratch[:, 0 : cn - half],
                        xb_t[0][:, c0 + half : c0 + cn],
                        mybir.ActivationFunctionType.Copy,
                        accum_out=pooled_t[0][:, 2 * d + 1 : 2 * d + 2],
                    )

            # pair-1 reduce pieces: 1024-col slices, 1 of 3 on DVE and 2 of 3
            # on ACT (DVE is the busier engine during conv), each placed via
            # the wait-ladder so none displaces the conv PSUM copies
            P1_PIECES = [(1024 * k, 1024) for k in range(15)] + [
                (15360, 512),
                (15872, 512),
            ]

            def emit_red1_pieces(pieces):
                for k in pieces:
                    c0, cn = P1_PIECES[k]
                    dst = pooled_t[1][:, k : k + 1]
                    with tc.tile_wait_until(PIECE_W0 + PIECE_DW * k):
                        if k % 3 == 0:
                            nc.vector.reduce_sum(
                                dst, xb_t[1][:, c0 : c0 + cn],
                                axis=mybir.AxisListType.X,
                            )
                        else:
                            nc.scalar.activation(
                                act_scratch[:, 0:cn],
                                xb_t[1][:, c0 : c0 + cn],
                                mybir.ActivationFunctionType.Copy,
                                accum_out=dst,
                            )

            def emit_tail_masked(p, npart):
                pooled = pooled_t[p]
                nc.vector.reduce_sum(
                    pooled[:, 17:18], pooled[:, 0:npart], axis=mybir.AxisListType.X
                )
                masked = spool.tile([128, 2], f32, tag="masked", name=f"masked_{p}")
                nc.vector.tensor_scalar_mul(masked[:], mask01, pooled[:, 17:18])
                return masked

            def emit_logits(p, masked):
                logits_ps = rps.tile([2, E], f32, tag="rps", name=f"lg_{p}")
                nc.tensor.matmul(logits_ps[:], masked[:], wrT_rep)
                return logits_ps

            def emit_bias(p, logits_ps):
                logits_sb = spool.tile([2, E], f32, tag="lsb", name=f"lsb_{p}")
                nc.vector.tensor_tensor(
                    logits_sb[:], logits_ps[:], bias_rep, mybir.AluOpType.add
                )
                return logits_sb

            def emit_sig(p, logits_sb):
                rT = spool.tile([2, E], f32, tag="rT", name=f"rT_{p}")
                nc.scalar.activation(
                    rT[:], logits_sb[:], mybir.ActivationFunctionType.Sigmoid
                )
                return rT

            def emit_bcast(p, rT):
                rbc_ps = rps.tile([128, E], f32, tag="rps", name=f"rb_{p}")
                nc.tensor.matmul(rbc_ps[:], mask2, rT[:])
                rs = spool.tile([128, E], f32, tag="rs", name=f"rs_{p}")
                nc.vector.tensor_copy(rs[:], rbc_ps[:])
                return rs

            def emit_mix(p, rs):
                # wmixT[c(+64h), tap*64+o] = sum_e r[h, e] * weT[., e, .] (bf16)
                mixa = mpool.tile([128, C * 9], bf16, tag="mixa", name=f"mixa_{p}")
                mixb = mpool.tile([128, C * 9], bf16, tag="mixb", name=f"mixb_{p}")
                nc.vector.tensor_scalar_mul(mixa[:], weT[:, 0:576], rs[:, 0:1])
                nc.vector.scalar_tensor_tensor(
                    mixb[:], weT[:, 576:1152], rs[:, 1:2], mixa[:],
                    op0=mybir.AluOpType.mult, op1=mybir.AluOpType.add,
                )
                nc.vector.scalar_tensor_tensor(
                    mixa[:], weT[:, 1152:1728], rs[:, 2:3], mixb[:],
                    op0=mybir.AluOpType.mult, op1=mybir.AluOpType.add,
                )
                nc.vector.scalar_tensor_tensor(
                    mixb[:], weT[:, 1728:2304], rs[:, 3:4], mixa[:],
                    op0=mybir.AluOpType.mult, op1=mybir.AluOpType.add,
                )
                return mixb

            # ---------------- pair-0 reduces + HAM warmup --------------------
            # Warmup group d is gated on pair-0 chunk d; alternating half-bank
            # outputs keep consecutive matmuls free of write-write deps so
            # they pipeline at N cycles each and never clog the PE FIFO.
            wu_ps = rps.tile([C, 512], f32, tag="rps", name="wu")
            for d in range(len(P0_CHUNKS)):
                emit_red0(d)
            for d, count in WU_ORDER:
                c0, _ = P0_CHUNKS[d]
                for k in range(count):
                    half = (k % 2) * 256
                    nc.tensor.matmul(
                        wu_ps[:, half : half + 256],
                        weT[:, 0:C],
                        xb_t[0][:, c0 + half : c0 + half + 256],
                    )

            # the pair-0 routing chain outranks everything else that becomes
            # ready at the same scheduler instant (esp. pair-1 reduce pieces)
            with tc.high_priority():
                masked0 = emit_tail_masked(0, 7)
                lsb0 = emit_bias(0, emit_logits(0, masked0))
                rs0 = emit_bcast(0, emit_sig(0, lsb0))
            # micro-warmups right behind the routing matmuls in the PE FIFO:
            # keep the HAM activity window fed until the conv starts
            wu2 = convps.tile([C, 512], f32, tag="cps", name="wu2")
            for k in range(6):
                half = (k % 2) * 256
                nc.tensor.matmul(
                    wu2[:, half : half + 256],
                    weT[:, 0:C],
                    xb_t[0][:, half : half + 256],
                )
            with tc.high_priority():
                wmixT_t = [emit_mix(0, rs0), None]

            # pair-1 prep: reduce pieces through the ladder, then the routing
            # tail + mix at the ladder's end (lands mid-conv-p0, well before
            # conv p1 needs wmixT_t[1])
            emit_red1_pieces(range(17))
            with tc.tile_wait_until(PIECE_W0 + PIECE_DW * 18):
                masked1 = emit_tail_masked(1, 17)
                lsb1 = emit_bias(1, emit_logits(1, masked1))
                rs1 = emit_bcast(1, emit_sig(1, lsb1))
                wmixT_t[1] = emit_mix(1, rs1)

            # ---------------- conv ----------------
            for p in range(NPAIR):
                conv_scope = nc.named_scope(f"conv_p{p}"); conv_scope.__enter__()
                xb = xb_t[p]
                xb3 = xb.rearrange("p_ (r c) -> p_ r c", c=W)
                for sg in range(NSG):
                    stA = stpool.tile(
                        [128, 4, 2, 512], bf16, tag="stage", name=f"stA_{p}_{sg}"
                    )
                    stB = stpool.tile(
                        [128, 4, 2, 512], bf16, tag="stage", name=f"stB_{p}_{sg}"
                    )
                    last_sg = p == NPAIR - 1 and sg == NSG - 1
                    for g4 in range(4):
                        for tg in range(2):
                            t = 8 * sg + 2 * g4 + tg
                            wmixT = wmixT_t[p]
                            psA = convps.tile(
                                [128, 512], f32, tag="cps", name=f"psA_{p}_{t}"
                            )
                            psB = convps.tile(
                                [128, 512], f32, tag="cps", name=f"psB_{p}_{t}"
                            )
                            psA3 = psA.rearrange("p_ (r c) -> p_ r c", c=W)
                            psB3 = psB.rearrange("p_ (r c) -> p_ r c", c=W)
                            # stream (h, q) -> psum region: (0,0)->psA[0:64],
                            # (1,1)->psA[64:], (1,0)->psB[0:64], (0,1)->psB[64:]
                            for tap_idx, (kh, kw) in enumerate(TAPS):
                                cstart = max(0, 1 - kw)
                                cend = min(W, W + 1 - kw)
                                ncols = cend - cstart
                                ic0 = cstart + kw - 1
                                for h in range(2):
                                    for q in range(2):
                                        ps3 = psA3 if h == q else psB3
                                        j = 2 * t + q
                                        rstart = max(4 * j, 1 - kh)
                                        rend = min(4 * j + 4, H + 1 - kh)
                                        nrows = rend - rstart
                                        ir0 = rstart + kh - 1
                                        nc.tensor.matmul(
                                            ps3[
                                                64 * q : 64 * q + 64,
                                                rstart - 4 * j : rstart - 4 * j + nrows,
                                                cstart:cend,
                                            ],
                                            wmixT[
                                                64 * h : 64 * h + 64,
                                                (3 * kh + kw) * 64
                                                : (3 * kh + kw) * 64 + 64,
                                            ],
                                            xb3[
                                                64 * h : 64 * h + 64,
                                                ir0 : ir0 + nrows,
                                                ic0 : ic0 + ncols,
                                            ],
                                            start=(tap_idx == 0),
                                            stop=(tap_idx == len(TAPS) - 1),
                                        )
                            # stA on ACT, stB on DVE (split so both keep up)
                            nc.scalar.copy(stA[:, g4, tg, :], psA[:])
                            nc.vector.tensor_copy(stB[:, g4, tg, :], psB[:])
                            if last_sg and g4 == 3:
                                # very last chunks: store per tg so the final
                                # DMA is small and the kernel tail shrinks
                                bA, bB = 2 * p, 2 * p + 1
                                nc.sync.dma_start(
                                    y_g[bA, :, sg, g4, tg, 0, :],
                                    stA[0:64, g4, tg, :],
                                )
                                nc.sync.dma_start(
                                    y_g[bA, :, sg, g4, tg, 1, :],
                                    stB[64:128, g4, tg, :],
                                )
                                nc.scalar.dma_start(
                                    y_g[bB, :, sg, g4, tg, 0, :],
                                    stB[0:64, g4, tg, :],
                                )
                                nc.scalar.dma_start(
                                    y_g[bB, :, sg, g4, tg, 1, :],
                                    stA[64:128, g4, tg, :],
                                )
                        # batched stores: two g4 groups per DMA (262KB), the
                        # final sg degrades to per-g4 / per-tg for a short tail
                        bA, bB = 2 * p, 2 * p + 1
                        if last_sg:
                            ranges = {1: (0, 2), 2: (2, 3)}.get(g4)
                        else:
                            ranges = {1: (0, 2), 3: (2, 4)}.get(g4)
                        if ranges is not None:
                            glo, ghi = ranges
                            nc.sync.dma_start(
                                y_g[bA, :, sg, glo:ghi, :, 0, :],
                                stA[0:64, glo:ghi, :, :],
                            )
                            nc.sync.dma_start(
                                y_g[bA, :, sg, glo:ghi, :, 1, :],
                                stB[64:128, glo:ghi, :, :],
                            )
                            nc.scalar.dma_start(
                                y_g[bB, :, sg, glo:ghi, :, 0, :],
                                stB[0:64, glo:ghi, :, :],
                            )
                            nc.scalar.dma_start(
                                y_g[bB, :, sg, glo:ghi, :, 1, :],
                                stA[64:128, glo:ghi, :, :],
                            )
                conv_scope.__exit__(None, None, None)

    nc.compile()
    return nc


def _run(inputs, trace=False, **kw):
    import ml_dtypes
    from concourse import bass_utils

    nc = _get_nc()
    x = np.asarray(inputs["x"])
    if x.dtype != ml_dtypes.bfloat16:
        x = np.ascontiguousarray(x, dtype=np.float32).astype(ml_dtypes.bfloat16)
    we = np.ascontiguousarray(inputs["w_experts"], dtype=np.float32)
    wexT = np.ascontiguousarray(
        we.transpose(2, 0, 3, 4, 1).reshape(C, E * 9 * C)
    ).astype(ml_dtypes.bfloat16)
    weT128 = np.ascontiguousarray(np.concatenate([wexT, wexT], axis=0))
    wr = np.ascontiguousarray(inputs["w_route"], dtype=np.float32)
    wrT = np.ascontiguousarray(wr.T * np.float32(1.0 / HW))
    br = np.ascontiguousarray(inputs["b_route"], dtype=np.float32)
    constsA = np.zeros((128, 6), dtype=np.float32)
    constsA[0:64, 0] = 1.0
    constsA[64:128, 1] = 1.0
    constsA[0:64, 2:6] = wrT
    constsA[64:128, 2:6] = wrT
    constsB = np.zeros((2, 132), dtype=np.float32)
    constsB[0, 0:64] = 1.0
    constsB[1, 64:128] = 1.0
    constsB[:, 128:132] = br[None, :]
    in_maps = [
        {
            "x": x[i * NS : (i + 1) * NS],
            "weT128": weT128,
            "constsA": constsA,
            "constsB": constsB,
        }
        for i in range(N_CORES)
    ]
    res = bass_utils.run_bass_kernel_spmd(
        nc, in_maps, core_ids=list(range(N_CORES)), trace=trace, **kw
    )
    y = np.concatenate(
        [np.asarray(res.results[i]["y"]).astype(np.float32) for i in range(N_CORES)],
        axis=0,
    )
    return y, res


def _get_nc():
    if "nc" not in _CACHE:
        _CACHE["nc"] = _build_nc()
    return _CACHE["nc"]


def kernel(**inputs):
    y, _ = _run(inputs)
    return y


# revision 30
# speedup vs baseline: 1.2525x; 1.2525x over previous
"""Dynamic (MoE-routed) 3x3 conv kernel for Trainium2, 8 NeuronCores.

Problem: nn_DynamicConv_670014898566
  x         [32, 64, 128, 128] f32
  w_route   [4, 64] f32
  b_route   [4] f32
  w_experts [4, 64, 64, 3, 3] f32
  y = per-sample conv2d(x, sigmoid(mean(x,HW) @ w_route.T + b_route) @ w_experts, SAME)

Sharding: data-parallel over batch, 4 samples per core (2 pairs of 2).

v4 design (vs the 127us baseline):
  - x is cast to bf16 on the host (numerically identical to the SDMA
    cast-DMA the baseline used; the conv consumed bf16 either way), so
    the device reads half the bytes
  - loads are split across BOTH HWDGE rings in parallel (each ring has
    a ~2us inter-DMA bubble; two rings overlap them and together reach
    the HBM ceiling)
  - constants are merged into 3 host-prepared DMAs (expert kernels
    pre-transposed/replicated to the full 128-partition lhsT layout,
    routing matrix pre-scaled and replicated, masks + bias packed)
  - per-chunk channel-sum reduces for pair 0 (DVE first half, ACT rest)
    fire as each chunk lands; ALL pair-1 reduces run on the otherwise
    idle GpSimd engine so they can never block the DVE/ACT pipelines
  - HAM warmup: pipelined dummy matmuls (alternating PSUM half-banks to
    avoid write-write serialization) gated on pair-0 chunks keep the PE
    at 2.4 GHz through conv start without clogging the PE FIFO
  - routing chain: tail sum -> masked cols -> logits matmul -> bias ->
    sigmoid -> broadcast matmul -> copy to SBUF -> 4-op DVE mix chain
  - conv: per (sample h, chunk-parity q) stream, 9 shifted bf16 matmuls
    accumulate into one PSUM region; 4-way PE tile parallelism
  - pair-1 routing tail spliced mid-conv-p0 with wide margins
  - y stored as bf16 in 262KB two-group DMAs (A-samples on sync ring,
    B-samples on scalar); host upcasts to f32
"""

import sys

sys.path.insert(0, "/opt/trn_rl_repo")

import numpy as np

B, C, H, W = 32, 64, 128, 128
E = 4
HW = H * W
N_CORES = 8
NS = B // N_CORES          # samples per core = 4
NPAIR = NS // 2            # pairs per core = 2
NT = 16                    # chunk-pairs (t) per pair
NSG = 2                    # store super-groups per pair (4 g of 2 t each)
# load chunks (cols of the [128, 16384] pair tile), all on the gpsimd
# SWDGE queue (the only path that streams near HBM rate; descriptor
# generation costs ~4.15us/DMA serially on Q7, so few chunks)
P0_CHUNKS = [(0, 6144), (6144, 6144), (12288, 3584), (15872, 512)]
P1_CHUNKS = [(0, 4096), (4096, 4096), (8192, 4096), (12288, 4096)]
WU_ORDER = [(0, 18), (1, 10), (2, 8), (3, 6)]  # HAM warmups per p0 chunk
# wait-ladder (ms, scheduler-sim time) spreading pair-1 reduce pieces and
# routing tail through the conv-p0 window so they never displace the
# PSUM-evacuation copies in the static engine queues
PIECE_W0 = 0.016
PIECE_DW = 0.0012
# full-coverage tap first (owns start=True so PSUM has_written covers the bank)
TAPS = [(1, 1), (0, 0), (0, 1), (0, 2), (1, 0), (1, 2), (2, 0), (2, 1), (2, 2)]

_CACHE = {}


def _build_nc():
    import concourse.bacc as bacc
    import concourse.mybir as mybir
    import concourse.tile as tile

    dt = mybir.dt
    f32 = dt.float32
    bf16 = dt.bfloat16

    nc = bacc.Bacc("TRN2", target_bir_lowering=False, debug=False, num_devices=N_CORES)

    x_d = nc.dram_tensor("x", [NS, C, H, W], bf16, kind="ExternalInput")
    # [128, 2304] bf16: lhsT expert kernels, host-replicated to both halves
    weT_d = nc.dram_tensor("weT128", [128, E * C * 9], bf16, kind="ExternalInput")
    # [128, 6] f32: cols 0:2 mask01, cols 2:6 wrT (pre-scaled, host-replicated)
    cA_d = nc.dram_tensor("constsA", [128, 6], f32, kind="ExternalInput")
    # [2, 132] f32: cols 0:128 mask2, cols 128:132 bias (both rows)
    cB_d = nc.dram_tensor("constsB", [2, 132], f32, kind="ExternalInput")
    y_d = nc.dram_tensor("y", [NS, C, H, W], bf16, kind="ExternalOutput")

    # x viewed as [(b c), (h w)]: pair p = rows 128p..128p+128
    x_flat = x_d.ap().rearrange("b c h w -> (b c) (h w)")
    # y viewed as [b, c, SG, g4, t2, parity, 4*W] for batched stores
    y_g = y_d.ap().rearrange(
        "b c (G g4 t2 hf r) w -> b c G g4 t2 hf (r w)", G=NSG, g4=4, t2=2, hf=2, r=4
    )

    with tile.TileContext(nc) as tc:
        with (
            tc.tile_pool(name="const", bufs=1) as cpool,
            tc.tile_pool(name="xp", bufs=2) as xpool,
            tc.tile_pool(name="mix", bufs=2) as mpool,
            tc.tile_pool(name="small", bufs=2) as spool,
            tc.tile_pool(name="stage", bufs=4) as stpool,
            tc.tile_pool(name="cps", bufs=7, space="PSUM") as convps,
            tc.tile_pool(name="rps", bufs=1, space="PSUM") as rps,
        ):
            xb_t = [
                xpool.tile([128, HW], bf16, tag="xt", name=f"xb_p{p}")
                for p in range(NPAIR)
            ]
            # pooled partials: p0 cols 0-8 (DVE/ACT per half-chunk), p1 cols
            # 0-16 (17 small pieces alternating DVE/ACT); col 17 = total
            pooled_t = [
                spool.tile([128, 18], f32, tag="pooled", name=f"pooled_{p}")
                for p in range(NPAIR)
            ]

            weT = cpool.tile([128, E * C * 9], bf16)
            constsA = cpool.tile([128, 6], f32)
            constsB = cpool.tile([2, 132], f32)
            mask01 = constsA[:, 0:2]
            wrT_rep = constsA[:, 2:6]
            mask2 = constsB[:, 0:128]
            bias_rep = constsB[:, 128:132]

            # ---------------- loads: SWDGE stream + one HWDGE chunk ----------
            def emit_load(p, c0, cn, eng=None):
                (eng or nc.gpsimd).dma_start(
                    xb_t[p][:, c0 : c0 + cn],
                    x_flat[128 * p : 128 * p + 128, c0 : c0 + cn],
                )

            for c0, cn in P0_CHUNKS:
                emit_load(0, c0, cn)
            for c0, cn in P1_CHUNKS:
                emit_load(1, c0, cn)
            # constants on the scalar ring (idle until B-stores)
            nc.scalar.dma_start(weT[:], weT_d.ap())
            nc.scalar.dma_start(constsA[:], cA_d.ap())
            nc.scalar.dma_start(constsB[:], cB_d.ap())
            # sigmoid table warm so the routing sigmoid isn't delayed later
            sig_warm = cpool.tile([1, 1], f32)
            nc.scalar.activation(
                sig_warm[:], constsB[0:1, 128:129],
                mybir.ActivationFunctionType.Sigmoid,
            )

            # ---------------- reduces ---------------------------------------
            act_scratch = cpool.tile([128, 3072], bf16)

            def emit_red0(d):
                """Pair-0 chunk d channel-sums: DVE first half, ACT the rest."""
                c0, cn = P0_CHUNKS[d]
                half = min((cn // 2 + 511) // 512 * 512, cn)
                nc.vector.reduce_sum(
                    pooled_t[0][:, 2 * d : 2 * d + 1],
                    xb_t[0][:, c0 : c0 + half],
                    axis=mybir.AxisListType.X,
                )
                if cn > half:
                    nc.scalar.activation(
                        act_scratch[:, 0 : cn - half],
                        xb_t[0][:, c0 + half : c0 + cn],
                        mybir.ActivationFunctionType.Copy,
                        accum_out=pooled_t[0][:, 2 * d + 1 : 2 * d + 2],
                    )

            # pair-1 reduce pieces: 1024-col slices, 1 of 3 on DVE and 2 of 3
            # on ACT (DVE is the busier engine during conv), each placed via
            # the wait-ladder so none displaces the conv PSUM copies
            P1_PIECES = [(1024 * k, 1024) for k in range(15)] + [
                (15360, 512),
                (15872, 512),
            ]

            def emit_red1_pieces(pieces):
                for k in pieces:
                    c0, cn = P1_PIECES[k]
                    dst = pooled_t[1][:, k : k + 1]
                    with tc.tile_wait_until(PIECE_W0 + PIECE_DW * k):
                        if k % 3 == 0:
                            nc.vector.reduce_sum(
                                dst, xb_t[1][:, c0 : c0 + cn],
                                axis=mybir.AxisListType.X,
                            )
                        else:
                            nc.scalar.activation(
                                act_scratch[:, 0:cn],
                                xb_t[1][:, c0 : c0 + cn],
                                mybir.ActivationFunctionType.Copy,
                                accum_out=dst,
                            )

            def emit_tail_masked(p, npart):
                pooled = pooled_t[p]
                nc.vector.reduce_sum(
                    pooled[:, 17:18], pooled[:, 0:npart], axis=mybir.AxisListType.X
                )
                masked = spool.tile([128, 2], f32, tag="masked", name=f"masked_{p}")
                nc.vector.tensor_scalar_mul(masked[:], mask01, pooled[:, 17:18])
                return masked

            def emit_logits(p, masked):
                logits_ps = rps.tile([2, E], f32, tag="rps", name=f"lg_{p}")
                nc.tensor.matmul(logits_ps[:], masked[:], wrT_rep)
                return logits_ps

            def emit_bias(p, logits_ps):
                logits_sb = spool.tile([2, E], f32, tag="lsb", name=f"lsb_{p}")
                nc.vector.tensor_tensor(
                    logits_sb[:], logits_ps[:], bias_rep, mybir.AluOpType.add
                )
                return logits_sb

            def emit_sig(p, logits_sb):
                rT = spool.tile([2, E], f32, tag="rT", name=f"rT_{p}")
                nc.scalar.activation(
                    rT[:], logits_sb[:], mybir.ActivationFunctionType.Sigmoid
                )
                return rT

            def emit_bcast(p, rT):
                rbc_ps = rps.tile([128, E], f32, tag="rps", name=f"rb_{p}")
                nc.tensor.matmul(rbc_ps[:], mask2, rT[:])
                rs = spool.tile([128, E], f32, tag="rs", name=f"rs_{p}")
                nc.vector.tensor_copy(rs[:], rbc_ps[:])
                return rs

            def emit_mix(p, rs):
                # wmixT[c(+64h), tap*64+o] = sum_e r[h, e] * weT[., e, .] (bf16)
                mixa = mpool.tile([128, C * 9], bf16, tag="mixa", name=f"mixa_{p}")
                mixb = mpool.tile([128, C * 9], bf16, tag="mixb", name=f"mixb_{p}")
                nc.vector.tensor_scalar_mul(mixa[:], weT[:, 0:576], rs[:, 0:1])
                nc.vector.scalar_tensor_tensor(
                    mixb[:], weT[:, 576:1152], rs[:, 1:2], mixa[:],
                    op0=mybir.AluOpType.mult, op1=mybir.AluOpType.add,
                )
                nc.vector.scalar_tensor_tensor(
                    mixa[:], weT[:, 1152:1728], rs[:, 2:3], mixb[:],
                    op0=mybir.AluOpType.mult, op1=mybir.AluOpType.add,
                )
                nc.vector.scalar_tensor_tensor(
                    mixb[:], weT[:, 1728:2304], rs[:, 3:4], mixa[:],
                    op0=mybir.AluOpType.mult, op1=mybir.AluOpType.add,
                )
                return mixb

            # ---------------- pair-0 reduces + HAM warmup --------------------
            # Warmup group d is gated on pair-0 chunk d; alternating half-bank
            # outputs keep consecutive matmuls free of write-write deps so
            # they pipeline at N cycles each and never clog the PE FIFO.
            wu_ps = rps.tile([C, 512], f32, tag="rps", name="wu")
            for d in range(len(P0_CHUNKS)):
                emit_red0(d)
            for d, count in WU_ORDER:
                c0, _ = P0_CHUNKS[d]
                for k in range(count):
                    half = (k % 2) * 256
                    nc.tensor.matmul(
                        wu_ps[:, half : half + 256],
                        weT[:, 0:C],
                        xb_t[0][:, c0 + half : c0 + half + 256],
                    )

            # the pair-0 routing chain outranks everything else that becomes
            # ready at the same scheduler instant (esp. pair-1 reduce pieces)
            with tc.high_priority():
                masked0 = emit_tail_masked(0, 7)
                lsb0 = emit_bias(0, emit_logits(0, masked0))
                rs0 = emit_bcast(0, emit_sig(0, lsb0))
            # micro-warmups right behind the routing matmuls in the PE FIFO:
            # keep the HAM activity window fed until the conv starts
            wu2 = convps.tile([C, 512], f32, tag="cps", name="wu2")
            for k in range(6):
                half = (k % 2) * 256
                nc.tensor.matmul(
                    wu2[:, half : half + 256],
                    weT[:, 0:C],
                    xb_t[0][:, half : half + 256],
                )
            with tc.high_priority():
                wmixT_t = [emit_mix(0, rs0), None]

            # pair-1 prep: reduce pieces through the ladder, then the routing
            # tail + mix at the ladder's end (lands mid-conv-p0, well before
            # conv p1 needs wmixT_t[1])
            emit_red1_pieces(range(17))
            with tc.tile_wait_until(PIECE_W0 + PIECE_DW * 18):
                masked1 = emit_tail_masked(1, 17)
                lsb1 = emit_bias(1, emit_logits(1, masked1))
                rs1 = emit_bcast(1, emit_sig(1, lsb1))
                wmixT_t[1] = emit_mix(1, rs1)

            # ---------------- conv ----------------
            for p in range(NPAIR):
                conv_scope = nc.named_scope(f"conv_p{p}"); conv_scope.__enter__()
                xb = xb_t[p]
                xb3 = xb.rearrange("p_ (r c) -> p_ r c", c=W)
                for sg in range(NSG):
                    stA = stpool.tile(
                        [128, 4, 2, 512], bf16, tag="stage", name=f"stA_{p}_{sg}"
                    )
                    stB = stpool.tile(
                        [128, 4, 2, 512], bf16, tag="stage", name=f"stB_{p}_{sg}"
                    )
                    last_sg = p == NPAIR - 1 and sg == NSG - 1
                    for g4 in range(4):
                        for tg in range(2):
                            t = 8 * sg + 2 * g4 + tg
                            wmixT = wmixT_t[p]
                            psA = convps.tile(
                                [128, 512], f32, tag="cps", name=f"psA_{p}_{t}"
                            )
                            psB = convps.tile(
                                [128, 512], f32, tag="cps", name=f"psB_{p}_{t}"
                            )
                            psA3 = psA.rearrange("p_ (r c) -> p_ r c", c=W)
                            psB3 = psB.rearrange("p_ (r c) -> p_ r c", c=W)
                            # stream (h, q) -> psum region: (0,0)->psA[0:64],
                            # (1,1)->psA[64:], (1,0)->psB[0:64], (0,1)->psB[64:]
                            for tap_idx, (kh, kw) in enumerate(TAPS):
                                cstart = max(0, 1 - kw)
                                cend = min(W, W + 1 - kw)
                                ncols = cend - cstart
                                ic0 = cstart + kw - 1
                                for h in range(2):
                                    for q in range(2):
                                        ps3 = psA3 if h == q else psB3
                                        j = 2 * t + q
                                        rstart = max(4 * j, 1 - kh)
                                        rend = min(4 * j + 4, H + 1 - kh)
                                        nrows = rend - rstart
                                        ir0 = rstart + kh - 1
                                        nc.tensor.matmul(
                                            ps3[
                                                64 * q : 64 * q + 64,
                                                rstart - 4 * j : rstart - 4 * j + nrows,
                                                cstart:cend,
                                            ],
                                            wmixT[
                                                64 * h : 64 * h + 64,
                                                (3 * kh + kw) * 64
                                                : (3 * kh + kw) * 64 + 64,
                                            ],
                                            xb3[
                                                64 * h : 64 * h + 64,
                                                ir0 : ir0 + nrows,
                                                ic0 : ic0 + ncols,
                                            ],
                                            start=(tap_idx == 0),
                                            stop=(tap_idx == len(TAPS) - 1),
                                        )
                            # stA on ACT, stB on DVE (split so both keep up)
                            nc.scalar.copy(stA[:, g4, tg, :], psA[:])
                            nc.vector.tensor_copy(stB[:, g4, tg, :], psB[:])
                            if last_sg and g4 == 3:
                                # very last chunks: store per tg so the final
                                # DMA is small and the kernel tail shrinks
                                bA, bB = 2 * p, 2 * p + 1
                                nc.sync.dma_start(
                                    y_g[bA, :, sg, g4, tg, 0, :],
                                    stA[0:64, g4, tg, :],
                                )
                                nc.sync.dma_start(
                                    y_g[bA, :, sg, g4, tg, 1, :],
                                    stB[64:128, g4, tg, :],
                                )
                                nc.scalar.dma_start(
                                    y_g[bB, :, sg, g4, tg, 0, :],
                                    stB[0:64, g4, tg, :],
                                )
                                nc.scalar.dma_start(
                                    y_g[bB, :, sg, g4, tg, 1, :],
                                    stA[64:128, g4, tg, :],
                                )
                        # batched stores: two g4 groups per DMA (262KB), the
                        # final sg degrades to per-g4 / per-tg for a short tail
                        bA, bB = 2 * p, 2 * p + 1
                        if last_sg:
                            ranges = {1: (0, 2), 2: (2, 3)}.get(g4)
                        else:
                            ranges = {1: (0, 2), 3: (2, 4)}.get(g4)
                        if ranges is not None:
                            glo, ghi = ranges
                            nc.sync.dma_start(
                                y_g[bA, :, sg, glo:ghi, :, 0, :],
                                stA[0:64, glo:ghi, :, :],
                            )
                            nc.sync.dma_start(
                                y_g[bA, :, sg, glo:ghi, :, 1, :],
                                stB[64:128, glo:ghi, :, :],
                            )
                            nc.scalar.dma_start(
                                y_g[bB, :, sg, glo:ghi, :, 0, :],
                                stB[0:64, glo:ghi, :, :],
                            )
                            nc.scalar.dma_start(
                                y_g[bB, :, sg, glo:ghi, :, 1, :],
                                stA[64:128, glo:ghi, :, :],
                            )
                conv_scope.__exit__(None, None, None)

    nc.compile()
    return nc


def _run(inputs, trace=False, **kw):
    import ml_dtypes
    from concourse import bass_utils

    nc = _get_nc()
    x = np.asarray(inputs["x"])
    if x.dtype != ml_dtypes.bfloat16:
        x = np.ascontiguousarray(x, dtype=np.float32).astype(ml_dtypes.bfloat16)
    we = np.ascontiguousarray(inputs["w_experts"], dtype=np.float32)
    wexT = np.ascontiguousarray(
        we.transpose(2, 0, 3, 4, 1).reshape(C, E * 9 * C)
    ).astype(ml_dtypes.bfloat16)
    weT128 = np.ascontiguousarray(np.concatenate([wexT, wexT], axis=0))
    wr = np.ascontiguousarray(inputs["w_route"], dtype=np.float32)
    wrT = np.ascontiguousarray(wr.T * np.float32(1.0 / HW))
    br = np.ascontiguousarray(inputs["b_route"], dtype=np.float32)
    constsA = np.zeros((128, 6), dtype=np.float32)
    constsA[0:64, 0] = 1.0
    constsA[64:128, 1] = 1.0
    constsA[0:64, 2:6] = wrT
    constsA[64:128, 2:6] = wrT
    constsB = np.zeros((2, 132), dtype=np.float32)
    constsB[0, 0:64] = 1.0
    constsB[1, 64:128] = 1.0
    constsB[:, 128:132] = br[None, :]
    in_maps = [
        {
            "x": x[i * NS : (i + 1) * NS],
            "weT128": weT128,
            "constsA": constsA,
            "constsB": constsB,
        }
        for i in range(N_CORES)
    ]
    res = bass_utils.run_bass_kernel_spmd(
        nc, in_maps, core_ids=list(range(N_CORES)), trace=trace, **kw
    )
    y = np.concatenate(
        [np.asarray(res.results[i]["y"]).astype(np.float32) for i in range(N_CORES)],
        axis=0,
    )
    return y, res


def _get_nc():
    if "nc" not in _CACHE:
        _CACHE["nc"] = _build_nc()
    return _CACHE["nc"]


def kernel(**inputs):
    y, _ = _run(inputs)
    return y
